# revision 47
# baseline (speedup 1.0000x reference)
"""Trainium2 Bass kernel for nn_Lowpass: EMA recurrence over time.

level_t = (1-s) * x_t + s * level_{t-1},  s = sigmoid(smoothing_var)

Strategy (v3):
  - Data-parallel over batch: 16 batches -> 8 cores x 2 batches.
  - Time in blocks of L=128 (partition dim).  Within a block the
    recurrence is a lower-triangular matmul; across blocks the carry
    is applied by a second accumulating matmul (mm2) whose stationary
    operand is the geometric column p, so ALL carry work rides on the
    otherwise-idle TensorE (float32r, full-rate fp32).
  - Block outputs are ROTATED by one row (stationary = roll(A,1)^T) so
    the carry row y[t0+127] lands on PSUM/SBUF partition 0 - a legal PE
    base partition for mm2's moving operand.  Stores: rows 1..127 go out
    as one [127 x 4KB] DMA per block; the four row-0s of a load group go
    out as a single [4 x 4KB] DMA (DRAM-side first dim 4), so the
    rotation costs only ~1/5 extra store time.
  - PSUM -> SBUF copies run mostly on DVE (no DMA queue of its own) with
    ~1/3 on ActE; x-loads and y-stores spread across all three DMA queues
    (sync/scalar/gpsimd), with the scalar queue kept free of bulk loads so
    ActE copies are not blocked.  The whole x shard is prefetched up front.
"""

import os
import sys
import functools

sys.path.insert(0, "/opt/trn_rl_repo")
os.environ.setdefault("MYCRO_LOCAL_CACHE", "1")

import numpy as np

B, T, U = 16, 2048, 1024
NCORES = 8
BL = B // NCORES          # batches per core
L = 128                   # time block == partition dim
NBLK = T // L             # 16 blocks per batch
GRP = 4                   # time blocks per x-load DMA / y group tile
NG = NBLK // GRP          # groups per batch
H = 512                   # matmul moving-free (one PSUM bank)
YG = 4                    # blocks per y group tile
NYG = NBLK // YG          # y groups per batch

# --- engine / queue assignment tables (tuned against CoreSim) ---------
# 6 big x-loads, order (g, b) for g=1..3 (group 0 is loaded per-block).
# The scalar queue carries no big loads so Act can absorb a few copies.
LOADQ = ["sync", "gpsimd", "gpsimd", "sync", "gpsimd", "sync"]
# 32 block stores (rows 1..127), order (g, n, b)
STOREQ = [
    "scalar", "sync", "scalar", "sync", "scalar", "scalar", "scalar", "gpsimd",
    "scalar", "sync", "scalar", "scalar", "sync", "sync", "sync", "gpsimd",
    "scalar", "sync", "scalar", "gpsimd", "gpsimd", "sync", "gpsimd", "scalar",
    "scalar", "gpsimd", "gpsimd", "sync", "gpsimd", "scalar", "scalar", "scalar",
]
# 8 row-0 group stores, order (yg, b)
STOREQ0 = ["gpsimd", "scalar", "scalar", "scalar", "gpsimd", "sync", "sync", "gpsimd"]
# 32 PSUM->SBUF copies, order (g, n, b): DVE with ten on Act
# (GPSIMD cannot read PSUM on hardware)
COPYE = ["vector"] * 32
for _i in (0, 2, 15, 21, 23, 24, 29):
    COPYE[_i] = "scalar"


G0Q = ("sync", "gpsimd", "sync", "gpsimd", "sync", "gpsimd", "sync", "gpsimd")


@functools.lru_cache(maxsize=4)
def _build(loadq: tuple, storeq: tuple, storeq0: tuple, copye: tuple,
           zero_c0: bool = False, g0q: tuple = G0Q):
    import concourse.tile as tile
    from concourse import bacc, mybir

    nc = bacc.Bacc("TRN2", target_bir_lowering=False, debug=False)
    f32 = mybir.dt.float32
    f32r = mybir.dt.float32r
    x = nc.dram_tensor("x", [BL, T, U], f32, kind="ExternalInput").ap()
    at = nc.dram_tensor("at", [L, L], f32, kind="ExternalInput").ap()
    pc = nc.dram_tensor("pc", [1, L], f32, kind="ExternalInput").ap()
    c0 = nc.dram_tensor("c0", [1, U], f32, kind="ExternalInput").ap()
    y = nc.dram_tensor("y", [BL, T, U], f32, kind="ExternalOutput").ap()

    # dram views: [b, g, p, n, u] so one group DMA fills SBUF [128, grp*U]
    xr = x.rearrange("b (g n p) u -> b g p n u", n=GRP, p=L)
    # [b, blk, p, u] for stores
    yv = y.rearrange("b (k p) u -> b k p u", p=L)

    with tile.TileContext(nc) as tc:
        with (
            tc.tile_pool(name="const", bufs=1) as constp,
            tc.tile_pool(name="xin0", bufs=GRP * BL) as xin0p,
            tc.tile_pool(name="xin", bufs=4) as xinp,
            tc.tile_pool(name="yout", bufs=6) as youtp,
            tc.tile_pool(name="ypsum", bufs=4, space="PSUM") as ypp,
        ):
            c0t = constp.tile([1, U], f32r)
            pct = constp.tile([1, L], f32r)
            att = constp.tile([L, L], f32r)

            prevc = [[c0t[0:1, h * H : (h + 1) * H] for h in range(2)]
                     for _ in range(BL)]
            # x loads all upfront (whole shard fits in SBUF); group 0
            # per-block so the carry chains start ~4us earlier
            xtiles = []
            xblk0 = [[None] * GRP for _ in range(BL)]
            ng0 = 0
            for b in range(BL):
                xb = xin0p.tile([L, U], f32r)
                getattr(nc, g0q[ng0]).dma_start(xb[:, :], xr[b, 0][:, 0].bitcast(f32r))
                ng0 += 1
                xblk0[b][0] = xb
            if not zero_c0:
                nc.scalar.dma_start(c0t[0:1, 0:H], c0[:, 0:H].bitcast(f32r))
                nc.scalar.dma_start(c0t[0:1, H:U], c0[:, H:U].bitcast(f32r))
            nc.scalar.dma_start(att[:, :], at.bitcast(f32r))
            nc.scalar.dma_start(pct[:, :], pc.bitcast(f32r))
            for n in range(1, GRP):
                for b in range(BL):
                    xb = xin0p.tile([L, U], f32r)
                    getattr(nc, g0q[ng0]).dma_start(
                        xb[:, :], xr[b, 0][:, n].bitcast(f32r)
                    )
                    ng0 += 1
                    xblk0[b][n] = xb
            nload = 0
            for g in range(1, NG):
                row = []
                for b in range(BL):
                    xt = xinp.tile([L, GRP * U], f32r)
                    xt3 = xt[:, :].rearrange("p (n u) -> p n u", n=GRP)
                    getattr(nc, loadq[nload]).dma_start(xt3, xr[b, g].bitcast(f32r))
                    nload += 1
                    row.append(xt)
                xtiles.append(row)

            nstore = ncopy = ngstore = 0
            ygs = [None] * BL
            for g in range(NG):
                for b in range(BL):
                    if (g * GRP) % YG == 0:
                        yg_t = youtp.tile([L, YG * U], f32r, name="yg_t")
                        ygs[b] = yg_t
                for n in range(GRP):
                    for b in range(BL):
                        if g == 0:
                            xap = xblk0[b][n][:, :]
                            xoff = 0
                        else:
                            xap = xtiles[g - 1][b][:, :]
                            xoff = n * U
                        yg = ygs[b]
                        k = g * GRP + n
                        m = k % YG             # slice within the y group
                        yp = ypp.tile([L, U], f32)
                        first = g == 0 and n == 0
                        for h in range(2):
                            sl = slice(h * H, (h + 1) * H)
                            if not (first and zero_c0):
                                # carry: yp[m] += p[m]*c  (p[m]=s^((m-1)%128+1))
                                nc.tensor.matmul(
                                    yp[:, sl], lhsT=pct[:, :], rhs=prevc[b][h],
                                    start=True, stop=False,
                                )
                            # local scan, rows rotated +1: yp[m] = y[t0+(m-1)%128]
                            nc.tensor.matmul(
                                yp[:, sl], lhsT=att[:, :],
                                rhs=xap[:, xoff + h * H : xoff + (h + 1) * H],
                                start=(first and zero_c0), stop=True,
                            )
                        ce = copye[ncopy]
                        ncopy += 1
                        if ce == "scalar":
                            nc.scalar.activation(
                                yg[:, m * U : (m + 1) * U], yp[:, :],
                                mybir.ActivationFunctionType.Copy,
                            )
                        else:
                            getattr(nc, ce).tensor_copy(
                                yg[:, m * U : (m + 1) * U], yp[:, :]
                            )
                        for h in range(2):
                            prevc[b][h] = yg[0:1, m * U + h * H : m * U + (h + 1) * H]
                        # rows 1..127 -> y[t0 .. t0+126]
                        getattr(nc, storeq[nstore]).dma_start(
                            yv[b, k][0 : L - 1],
                            yg[1:L, m * U : (m + 1) * U].bitcast(f32),
                        )
                        nstore += 1
                if (g * GRP + GRP) % YG == 0:
                    for b in range(BL):
                        # the YG row-0s of this y group -> y rows t0_m + 127
                        k1 = g * GRP + GRP
                        rows127 = yv[b][k1 - YG : k1, L - 1]  # [YG, U]
                        sb = ygs[b][0:1, :].rearrange("p (m u) -> p m u", m=YG)
                        getattr(nc, storeq0[ngstore]).dma_start(
                            rows127, sb.bitcast(f32)
                        )
                        ngstore += 1
    nc.compile()
    return nc


def _host_params(smoothing_var: np.ndarray):
    """s (fp32, as the reference computes it)."""
    sm = smoothing_var.astype(np.float32).reshape(-1)
    return (1.0 / (1.0 + np.exp(-sm.astype(np.float64)))).astype(np.float32)


def _host_mats(s32_scalar):
    """Stationary matrices: roll(A,1)^T (rows rotated +1) and p column."""
    s = np.float64(s32_scalar)
    j = np.arange(L)[:, None]
    i = np.arange(L)[None, :]
    A = np.where(j >= i, (1.0 - s) * s ** (j - i), 0.0)
    Arot = np.roll(A, 1, axis=0)          # PSUM row m = y[t0 + (m-1)%128]
    AT = np.ascontiguousarray(Arot.T.astype(np.float32))
    m = np.arange(L)
    pcol = (s ** (((m - 1) % L) + 1)).astype(np.float32).reshape(1, L)
    return AT, np.ascontiguousarray(pcol)


def kernel(inputs: np.ndarray, level_var: np.ndarray, smoothing_var: np.ndarray):
    from concourse import bass_utils

    x = np.ascontiguousarray(inputs, dtype=np.float32)
    assert x.shape == (B, T, U), x.shape
    s32 = _host_params(smoothing_var)
    if not np.all(s32 == s32[0]):
        # general per-unit s: fall back to exact numpy recurrence
        return _kernel_numpy(x, level_var, s32)
    AT, pcol = _host_mats(s32[0])
    c0 = np.ascontiguousarray(level_var.astype(np.float32).reshape(1, U))
    zero_c0 = bool(np.all(c0 == 0.0))

    nc = _build(tuple(LOADQ), tuple(STOREQ), tuple(STOREQ0), tuple(COPYE), zero_c0)
    in_maps = [
        {"x": np.ascontiguousarray(x[c * BL : (c + 1) * BL]), "at": AT,
         "pc": pcol, "c0": c0}
        for c in range(NCORES)
    ]
    res = bass_utils.run_bass_kernel_spmd(nc, in_maps, core_ids=list(range(NCORES)))
    out = np.concatenate([res.results[c]["y"] for c in range(NCORES)], axis=0)
    return out


def _kernel_numpy(x, level_var, s32):
    out = np.empty_like(x)
    c = np.broadcast_to(level_var.reshape(1, U), (x.shape[0], U)).astype(np.float32)
    one_minus = (1.0 - s32).astype(np.float32)
    for t in range(x.shape[1]):
        c = one_minus * x[:, t] + s32 * c
        out[:, t] = c
    return out


if __name__ == "__main__":
    rng = np.random.default_rng(0)
    xs = rng.standard_normal((B, T, U)).astype(np.float32)
    e = np.exp(-0.001 / 0.1)
    sm = np.full((1, U), np.log(e / (1 - e)), np.float32)
    lv = np.zeros((1, U), np.float32)
    o = kernel(xs, lv, sm)
    print("out", o.shape, o.dtype, float(np.abs(o).max()))


# revision 48
# speedup vs baseline: 1.0032x; 1.0032x over previous
"""Trainium2 Bass kernel for nn_Lowpass: EMA recurrence over time.

level_t = (1-s) * x_t + s * level_{t-1},  s = sigmoid(smoothing_var)

Strategy (v3):
  - Data-parallel over batch: 16 batches -> 8 cores x 2 batches.
  - Time in blocks of L=128 (partition dim).  Within a block the
    recurrence is a lower-triangular matmul; across blocks the carry
    is applied by a second accumulating matmul (mm2) whose stationary
    operand is the geometric column p, so ALL carry work rides on the
    otherwise-idle TensorE (float32r, full-rate fp32).
  - Block outputs are ROTATED by one row (stationary = roll(A,1)^T) so
    the carry row y[t0+127] lands on PSUM/SBUF partition 0 - a legal PE
    base partition for mm2's moving operand.  Stores: rows 1..127 go out
    as one [127 x 4KB] DMA per block; the four row-0s of a load group go
    out as a single [4 x 4KB] DMA (DRAM-side first dim 4), so the
    rotation costs only ~1/5 extra store time.
  - PSUM -> SBUF copies run mostly on DVE (no DMA queue of its own) with
    ~1/3 on ActE; x-loads and y-stores spread across all three DMA queues
    (sync/scalar/gpsimd), with the scalar queue kept free of bulk loads so
    ActE copies are not blocked.  The whole x shard is prefetched up front.
"""

import os
import sys
import functools

sys.path.insert(0, "/opt/trn_rl_repo")
os.environ.setdefault("MYCRO_LOCAL_CACHE", "1")

import numpy as np

B, T, U = 16, 2048, 1024
NCORES = 8
BL = B // NCORES          # batches per core
L = 128                   # time block == partition dim
NBLK = T // L             # 16 blocks per batch
GRP = 4                   # time blocks per x-load DMA / y group tile
NG = NBLK // GRP          # groups per batch
H = 512                   # matmul moving-free (one PSUM bank)
YG = 4                    # blocks per y group tile
NYG = NBLK // YG          # y groups per batch

# --- engine / queue assignment tables (tuned against CoreSim) ---------
# 6 big x-loads, order (g, b) for g=1..3 (group 0 is loaded per-block).
# The scalar queue carries no big loads so Act can absorb a few copies.
LOADQ = ["sync", "gpsimd", "gpsimd", "sync", "gpsimd", "sync"]
# 32 block stores (rows 1..127), order (g, n, b)
STOREQ = [
    "scalar", "sync", "scalar", "sync", "scalar", "scalar", "scalar", "gpsimd",
    "scalar", "sync", "scalar", "scalar", "sync", "sync", "sync", "gpsimd",
    "scalar", "sync", "scalar", "gpsimd", "gpsimd", "sync", "sync", "scalar",
    "scalar", "gpsimd", "scalar", "gpsimd", "sync", "gpsimd", "scalar", "scalar",
]
# 8 row-0 group stores, order (yg, b)
STOREQ0 = ["gpsimd", "scalar", "scalar", "scalar", "gpsimd", "gpsimd", "sync", "gpsimd"]
# 32 PSUM->SBUF copies, order (g, n, b): DVE with ten on Act
# (GPSIMD cannot read PSUM on hardware)
COPYE = ["vector"] * 32
for _i in (0, 2, 15, 21, 22, 23, 24, 29):
    COPYE[_i] = "scalar"


G0Q = ("sync", "gpsimd", "sync", "gpsimd", "sync", "gpsimd", "sync", "gpsimd")


@functools.lru_cache(maxsize=4)
def _build(loadq: tuple, storeq: tuple, storeq0: tuple, copye: tuple,
           zero_c0: bool = False, g0q: tuple = G0Q):
    import concourse.tile as tile
    from concourse import bacc, mybir

    nc = bacc.Bacc("TRN2", target_bir_lowering=False, debug=False)
    f32 = mybir.dt.float32
    f32r = mybir.dt.float32r
    x = nc.dram_tensor("x", [BL, T, U], f32, kind="ExternalInput").ap()
    at = nc.dram_tensor("at", [L, L], f32, kind="ExternalInput").ap()
    pc = nc.dram_tensor("pc", [1, L], f32, kind="ExternalInput").ap()
    c0 = nc.dram_tensor("c0", [1, U], f32, kind="ExternalInput").ap()
    y = nc.dram_tensor("y", [BL, T, U], f32, kind="ExternalOutput").ap()

    # dram views: [b, g, p, n, u] so one group DMA fills SBUF [128, grp*U]
    xr = x.rearrange("b (g n p) u -> b g p n u", n=GRP, p=L)
    # [b, blk, p, u] for stores
    yv = y.rearrange("b (k p) u -> b k p u", p=L)

    with tile.TileContext(nc) as tc:
        with (
            tc.tile_pool(name="const", bufs=1) as constp,
            tc.tile_pool(name="xin0", bufs=GRP * BL) as xin0p,
            tc.tile_pool(name="xin", bufs=4) as xinp,
            tc.tile_pool(name="yout", bufs=6) as youtp,
            tc.tile_pool(name="ypsum", bufs=4, space="PSUM") as ypp,
        ):
            c0t = constp.tile([1, U], f32r)
            pct = constp.tile([1, L], f32r)
            att = constp.tile([L, L], f32r)

            prevc = [[c0t[0:1, h * H : (h + 1) * H] for h in range(2)]
                     for _ in range(BL)]
            # x loads all upfront (whole shard fits in SBUF); group 0
            # per-block so the carry chains start ~4us earlier
            xtiles = []
            xblk0 = [[None] * GRP for _ in range(BL)]
            ng0 = 0
            for b in range(BL):
                xb = xin0p.tile([L, U], f32r)
                getattr(nc, g0q[ng0]).dma_start(xb[:, :], xr[b, 0][:, 0].bitcast(f32r))
                ng0 += 1
                xblk0[b][0] = xb
            if not zero_c0:
                nc.scalar.dma_start(c0t[0:1, 0:H], c0[:, 0:H].bitcast(f32r))
                nc.scalar.dma_start(c0t[0:1, H:U], c0[:, H:U].bitcast(f32r))
            nc.scalar.dma_start(att[:, :], at.bitcast(f32r))
            nc.scalar.dma_start(pct[:, :], pc.bitcast(f32r))
            for n in range(1, GRP):
                for b in range(BL):
                    xb = xin0p.tile([L, U], f32r)
                    getattr(nc, g0q[ng0]).dma_start(
                        xb[:, :], xr[b, 0][:, n].bitcast(f32r)
                    )
                    ng0 += 1
                    xblk0[b][n] = xb
            nload = 0
            for g in range(1, NG):
                row = []
                for b in range(BL):
                    xt = xinp.tile([L, GRP * U], f32r)
                    xt3 = xt[:, :].rearrange("p (n u) -> p n u", n=GRP)
                    getattr(nc, loadq[nload]).dma_start(xt3, xr[b, g].bitcast(f32r))
                    nload += 1
                    row.append(xt)
                xtiles.append(row)

            nstore = ncopy = ngstore = 0
            ygs = [None] * BL
            for g in range(NG):
                for b in range(BL):
                    if (g * GRP) % YG == 0:
                        yg_t = youtp.tile([L, YG * U], f32r, name="yg_t")
                        ygs[b] = yg_t
                for n in range(GRP):
                    for b in range(BL):
                        if g == 0:
                            xap = xblk0[b][n][:, :]
                            xoff = 0
                        else:
                            xap = xtiles[g - 1][b][:, :]
                            xoff = n * U
                        yg = ygs[b]
                        k = g * GRP + n
                        m = k % YG             # slice within the y group
                        yp = ypp.tile([L, U], f32)
                        first = g == 0 and n == 0
                        for h in range(2):
                            sl = slice(h * H, (h + 1) * H)
                            if not (first and zero_c0):
                                # carry: yp[m] += p[m]*c  (p[m]=s^((m-1)%128+1))
                                nc.tensor.matmul(
                                    yp[:, sl], lhsT=pct[:, :], rhs=prevc[b][h],
                                    start=True, stop=False,
                                )
                            # local scan, rows rotated +1: yp[m] = y[t0+(m-1)%128]
                            nc.tensor.matmul(
                                yp[:, sl], lhsT=att[:, :],
                                rhs=xap[:, xoff + h * H : xoff + (h + 1) * H],
                                start=(first and zero_c0), stop=True,
                            )
                        ce = copye[ncopy]
                        ncopy += 1
                        if ce == "scalar":
                            nc.scalar.activation(
                                yg[:, m * U : (m + 1) * U], yp[:, :],
                                mybir.ActivationFunctionType.Copy,
                            )
                        else:
                            getattr(nc, ce).tensor_copy(
                                yg[:, m * U : (m + 1) * U], yp[:, :]
                            )
                        for h in range(2):
                            prevc[b][h] = yg[0:1, m * U + h * H : m * U + (h + 1) * H]
                        # rows 1..127 -> y[t0 .. t0+126]
                        getattr(nc, storeq[nstore]).dma_start(
                            yv[b, k][0 : L - 1],
                            yg[1:L, m * U : (m + 1) * U].bitcast(f32),
                        )
                        nstore += 1
                if (g * GRP + GRP) % YG == 0:
                    for b in range(BL):
                        # the YG row-0s of this y group -> y rows t0_m + 127
                        k1 = g * GRP + GRP
                        rows127 = yv[b][k1 - YG : k1, L - 1]  # [YG, U]
                        sb = ygs[b][0:1, :].rearrange("p (m u) -> p m u", m=YG)
                        getattr(nc, storeq0[ngstore]).dma_start(
                            rows127, sb.bitcast(f32)
                        )
                        ngstore += 1
    nc.compile()
    return nc


def _host_params(smoothing_var: np.ndarray):
    """s (fp32, as the reference computes it)."""
    sm = smoothing_var.astype(np.float32).reshape(-1)
    return (1.0 / (1.0 + np.exp(-sm.astype(np.float64)))).astype(np.float32)


def _host_mats(s32_scalar):
    """Stationary matrices: roll(A,1)^T (rows rotated +1) and p column."""
    s = np.float64(s32_scalar)
    j = np.arange(L)[:, None]
    i = np.arange(L)[None, :]
    A = np.where(j >= i, (1.0 - s) * s ** (j - i), 0.0)
    Arot = np.roll(A, 1, axis=0)          # PSUM row m = y[t0 + (m-1)%128]
    AT = np.ascontiguousarray(Arot.T.astype(np.float32))
    m = np.arange(L)
    pcol = (s ** (((m - 1) % L) + 1)).astype(np.float32).reshape(1, L)
    return AT, np.ascontiguousarray(pcol)


def kernel(inputs: np.ndarray, level_var: np.ndarray, smoothing_var: np.ndarray):
    from concourse import bass_utils

    x = np.ascontiguousarray(inputs, dtype=np.float32)
    assert x.shape == (B, T, U), x.shape
    s32 = _host_params(smoothing_var)
    if not np.all(s32 == s32[0]):
        # general per-unit s: fall back to exact numpy recurrence
        return _kernel_numpy(x, level_var, s32)
    AT, pcol = _host_mats(s32[0])
    c0 = np.ascontiguousarray(level_var.astype(np.float32).reshape(1, U))
    zero_c0 = bool(np.all(c0 == 0.0))

    nc = _build(tuple(LOADQ), tuple(STOREQ), tuple(STOREQ0), tuple(COPYE), zero_c0)
    in_maps = [
        {"x": np.ascontiguousarray(x[c * BL : (c + 1) * BL]), "at": AT,
         "pc": pcol, "c0": c0}
        for c in range(NCORES)
    ]
    res = bass_utils.run_bass_kernel_spmd(nc, in_maps, core_ids=list(range(NCORES)))
    out = np.concatenate([res.results[c]["y"] for c in range(NCORES)], axis=0)
    return out


def _kernel_numpy(x, level_var, s32):
    out = np.empty_like(x)
    c = np.broadcast_to(level_var.reshape(1, U), (x.shape[0], U)).astype(np.float32)
    one_minus = (1.0 - s32).astype(np.float32)
    for t in range(x.shape[1]):
        c = one_minus * x[:, t] + s32 * c
        out[:, t] = c
    return out


if __name__ == "__main__":
    rng = np.random.default_rng(0)
    xs = rng.standard_normal((B, T, U)).astype(np.float32)
    e = np.exp(-0.001 / 0.1)
    sm = np.full((1, U), np.log(e / (1 - e)), np.float32)
    lv = np.zeros((1, U), np.float32)
    o = kernel(xs, lv, sm)
    print("out", o.shape, o.dtype, float(np.abs(o).max()))
